# revision 11
# baseline (speedup 1.0000x reference)
"""DIN attention layer kernel for Trainium2 (8 NeuronCores, data-parallel over batch).

Reference computation (per batch b):
    att = [q, k, q-k, q*k]            # [T, 4M]
    h1  = relu(att @ W1 + b1)         # [T, D]
    h2  = relu(h1 @ W2 + b2)          # [T, D]
    s   = h2 @ w_score + b_score      # [T, 1]
    attn = softmax(s.T + mask * -1e9) # [1, T]
    out = attn @ values               # [1, D]

Key optimizations over the straightforward mapping:
  * Data-parallel: 8 batches per core (B=64 over 8 cores).
  * Algebraic reassociation of the concat matmul:
        att @ W1 = q@(W1a+W1c) + k@(W1b-W1c+diag(q)@W1d)
    The q term is a per-batch row vector folded into the layer-1 bias (rt);
    diag(q)@W1d is folded into the layer-1 weight per batch (w1eff, two DVE
    FMAs), so the mm1 contraction is 256 instead of 1024 (mm1 quartered).
  * Transposed-activation layout [feature, token]: W1/W2 are used as stored
    for lhsT; only keys need a transpose, done on the PE via bf16 identity
    matmul (keys pre-cast to bf16; 4 transposes share one PSUM tile).
  * bf16 matmuls (fp32 PSUM accumulation); biases/softmax kept in fp32.
  * score = w_score . h2 and out = attn @ values are computed as
    per-partition FMA chains on the Vector engine (scalar_tensor_tensor)
    followed by a single ones-vector reduction matmul, instead of
    streaming h2/values through the PE a second time.
  * Softmax without max-subtraction (scores are O(1); masked lanes are
    exp(-1e9) = 0), sum fused into the Exp activation via accum_out.
  * Software pipelined emission: mm1 runs one batch ahead of mm2; the
    deferred attn@values chain for batch b runs during batch b+1's mm2.
"""

import numpy as np

P = 128
B = 8          # batches per core
T = 1024       # tokens
M = 256        # key feature dim
D = 1024       # hidden dim
MC = M // P    # key-feature chunks (2)
DC = D // P    # hidden chunks (8)
TC = T // P    # token chunks (8)
NH = 2         # free-dim halves of 512
NEG = -1.0e9

_built = None


def _ns(h):
    return slice(h * 512, (h + 1) * 512)


def _build():
    import concourse.bass as bass
    import concourse.bacc as bacc
    import concourse.mybir as mybir
    import concourse.tile as tile
    from concourse.masks import make_identity
    from contextlib import ExitStack

    F32 = mybir.dt.float32
    BF16 = mybir.dt.bfloat16
    AF = mybir.ActivationFunctionType
    OP = mybir.AluOpType

    nc = bacc.Bacc("TRN2")
    q_d = nc.dram_tensor("query", [B, M], F32, kind="ExternalInput").ap()
    k_d = nc.dram_tensor("keys", [B, T, M], F32, kind="ExternalInput").ap()
    v_d = nc.dram_tensor("values", [B, T, D], F32, kind="ExternalInput").ap()
    m_d = nc.dram_tensor("mask", [B, T], F32, kind="ExternalInput").ap()
    w1_d = nc.dram_tensor("W1", [4 * M, D], F32, kind="ExternalInput").ap()
    b1_d = nc.dram_tensor("b1", [D], F32, kind="ExternalInput").ap()
    w2_d = nc.dram_tensor("W2", [D, D], F32, kind="ExternalInput").ap()
    b2_d = nc.dram_tensor("b2", [D], F32, kind="ExternalInput").ap()
    ws_d = nc.dram_tensor("w_score", [D, 1], F32, kind="ExternalInput").ap()
    out_d = nc.dram_tensor("out", [B, D], F32, kind="ExternalOutput").ap()

    with tile.TileContext(nc) as tc, ExitStack() as ctx:
        cons = ctx.enter_context(tc.tile_pool(name="cons", bufs=1))
        stage = ctx.enter_context(tc.tile_pool(name="stage", bufs=4))
        kbf = ctx.enter_context(tc.tile_pool(name="kbf", bufs=2))
        xpool = ctx.enter_context(tc.tile_pool(name="xp", bufs=2))
        wep = ctx.enter_context(tc.tile_pool(name="wep", bufs=2))
        h1pool = ctx.enter_context(tc.tile_pool(name="h1p", bufs=2))
        h2pool = ctx.enter_context(tc.tile_pool(name="h2p", bufs=3))
        vpool = ctx.enter_context(tc.tile_pool(name="vp", bufs=1))
        vbpool = ctx.enter_context(tc.tile_pool(name="vbp", bufs=1))
        accp = ctx.enter_context(tc.tile_pool(name="accp", bufs=4))
        small = ctx.enter_context(tc.tile_pool(name="small", bufs=2))
        small1 = ctx.enter_context(tc.tile_pool(name="small1", bufs=1))
        dram = ctx.enter_context(tc.tile_pool(name="dram", bufs=2, space="DRAM"))
        psum_mm = ctx.enter_context(tc.tile_pool(name="psmm", bufs=4, space="PSUM"))
        psum_sc = ctx.enter_context(tc.tile_pool(name="pssc", bufs=2, space="PSUM"))
        psum_out = ctx.enter_context(tc.tile_pool(name="psout", bufs=2, space="PSUM"))

        # ---- constants ------------------------------------------------------
        identity = cons.tile([P, P], BF16)
        make_identity(nc, identity)
        ones_col = cons.tile([P, 1], BF16)
        nc.vector.memset(ones_col, 1.0)

        # ---- bulk input DMA issue, priority order (sync queue) --------------
        # keys b0/b1 first (feed the first PE work), then W1 pieces
        # (c, b, d gate mm1; a gates the rt bias), then W2, values b0, keys b2.
        kst = {}
        kst[0] = stage.tile([P, TC, M], F32, tag="stage", name="kst0")
        nc.sync.dma_start(kst[0], k_d[0].rearrange("(to p) m -> p to m", p=P))
        tmp_c = stage.tile([P, MC, D], F32, tag="stage", name="w1ct")
        nc.sync.dma_start(
            tmp_c, w1_d[2 * M:3 * M, :].rearrange("(c p) d -> p c d", p=P)
        )
        tmp_a = stage.tile([P, MC, D], F32, tag="stage", name="w1at")
        nc.sync.dma_start(tmp_a, w1_d[0:M, :].rearrange("(c p) d -> p c d", p=P))
        tmp_b = stage.tile([P, MC, D], F32, tag="stage", name="w1bt")
        nc.sync.dma_start(tmp_b, w1_d[M:2 * M, :].rearrange("(c p) d -> p c d", p=P))
        kst[1] = stage.tile([P, TC, M], F32, tag="stage", name="kst1")
        nc.sync.dma_start(kst[1], k_d[1].rearrange("(to p) m -> p to m", p=P))
        tmp_d = stage.tile([P, MC, D], F32, tag="stage", name="w1dt")
        nc.sync.dma_start(
            tmp_d, w1_d[3 * M:4 * M, :].rearrange("(c p) d -> p c d", p=P)
        )
        mask_t = {}
        for b in range(2):
            mask_t[b] = small.tile([1, T], F32, tag="mask", name=f"mask{b}")
            nc.sync.dma_start(mask_t[b], m_d[b:b + 1, :])

        # striped per-channel vectors via gpsimd (software DGE, parallel queue)
        qt_f = cons.tile([P, MC, B], F32)
        for c in range(MC):
            nc.gpsimd.dma_start(
                qt_f[:, c, :], q_d[:, c * P:(c + 1) * P].rearrange("b p -> p b")
            )
        b1_sb = cons.tile([P, DC], F32)
        nc.gpsimd.dma_start(b1_sb, b1_d.rearrange("(c p) -> p c", p=P))
        b2_sb = cons.tile([P, DC], F32)
        nc.gpsimd.dma_start(b2_sb, b2_d.rearrange("(c p) -> p c", p=P))
        ws_f = cons.tile([P, DC], F32)
        nc.gpsimd.dma_start(ws_f, ws_d.rearrange("(c p) one -> p (c one)", p=P))
        ws_sb = cons.tile([P, DC], BF16)
        nc.vector.tensor_copy(ws_sb, ws_f)

        # keys b0 cast (first DVE op so transposes can start immediately)
        keys_bf = {}
        keys_bf[0] = kbf.tile([P, TC, M], BF16, tag="kb", name="kbf0")
        nc.vector.tensor_copy(keys_bf[0], kst[0])

        # ---- per-batch stage emitters ---------------------------------------

        def emit_S1_transp(b):
            """keys^T via the DMA xbar: x_t[p, c, t] = keys[b, t, c*128+p]."""
            x_t = xpool.tile([P, MC, T], BF16, tag="X", name=f"x{b}")
            for c in range(MC):
                for to in range(TC):
                    nc.sync.dma_start_transpose(
                        x_t[:, c, to * P:(to + 1) * P],
                        keys_bf[b][:, to, c * P:(c + 1) * P],
                    )
            return x_t

        def emit_S1_weff(b):
            """w1eff_b = (W1b - W1c) + q_b * W1d  (per-batch layer-1 weight)."""
            we = wep.tile([P, MC, D], BF16, tag="we", name=f"we{b}")
            for c in range(MC):
                nc.vector.scalar_tensor_tensor(
                    we[:, c, :], in0=w1d_sb[:, c, :], scalar=qt_f[:, c, b:b + 1],
                    in1=w1bc[:, c, :], op0=OP.mult, op1=OP.add,
                )
            return we

        def emit_S2_mm1(b, x_t, we):
            """h1 = relu(w1eff^T @ x + rt_b)  -> [P, DC, T] bf16."""
            h1 = h1pool.tile([P, DC, T], BF16, tag="H1", name=f"h1_{b}")
            for j in range(DC):
                for h in range(NH):
                    ps = psum_mm.tile([P, 512], F32, tag="mm")
                    for c in range(MC):
                        nc.tensor.matmul(
                            ps, we[:, c, j * P:(j + 1) * P], x_t[:, c, _ns(h)],
                            start=(c == 0), stop=(c == MC - 1),
                        )
                    nc.scalar.activation(
                        h1[:, j, _ns(h)], ps, AF.Relu, bias=rt[:, b, j:j + 1],
                        scale=1.0,
                    )
            return h1

        carry = {}

        def emit_attnv_pe(b):
            """out = (attn_t^T @ vals) * rec on the PE (16 bf16 matmuls)."""
            st = carry.pop(b)
            out_sb = small1.tile([1, D], F32, tag="osb", name=f"osb{b}")
            ops = [
                psum_out.tile([1, 512], F32, tag="po", name=f"ops{b}_{h}")
                for h in range(NH)
            ]
            for h in range(NH):
                for c in range(TC):
                    nc.tensor.matmul(
                        ops[h], st["attn"][:, c:c + 1], st["vals"][:, c, _ns(h)],
                        start=(c == 0), stop=(c == TC - 1),
                    )
            for h in range(NH):
                nc.vector.tensor_scalar_mul(out_sb[:, _ns(h)], ops[h], st["rec"])
            nc.sync.dma_start(out_d[b:b + 1, :], out_sb)

        # ---- preamble: transposes, rt, then mm1 for batches 0 and 1 ---------
        x0 = emit_S1_transp(0)

        # rt[p, b, j] = (q_b @ (W1a+W1c))[j*128+p] + b1[j*128+p]; MUST be
        # fully emitted before any h1 activation that reads it.
        w1qc = cons.tile([P, MC, D], BF16)   # W1a + W1c
        nc.vector.tensor_add(w1qc, tmp_a, tmp_c)
        qt_b = cons.tile([P, MC, B], BF16)
        nc.vector.tensor_copy(qt_b, qt_f)
        rt = cons.tile([P, B, DC], F32)
        for j in range(DC):
            rt_ps = psum_sc.tile([P, B], F32, tag="sc", name=f"rtps{j}")
            for c in range(MC):
                nc.tensor.matmul(
                    rt_ps, w1qc[:, c, j * P:(j + 1) * P], qt_b[:, c, :],
                    start=(c == 0), stop=(c == MC - 1),
                )
            nc.vector.tensor_scalar(
                rt[:, :, j], rt_ps, b1_sb[:, j:j + 1], None, op0=OP.add
            )

        # keys b1 cast + transposes
        keys_bf[1] = kbf.tile([P, TC, M], BF16, tag="kb", name="kbf1")
        nc.vector.tensor_copy(keys_bf[1], kst[1])
        x1 = emit_S1_transp(1)

        # W1 combos for mm1 (DVE; ordered by DMA arrival)
        w1bc = cons.tile([P, MC, D], BF16)   # W1b - W1c
        nc.vector.tensor_sub(w1bc, tmp_b, tmp_c)
        w1d_sb = cons.tile([P, MC, D], BF16)  # W1d
        nc.vector.tensor_copy(w1d_sb, tmp_d)
        we0 = emit_S1_weff(0)
        we1 = emit_S1_weff(1)

        h1_t = {}
        h1_t[0] = emit_S2_mm1(0, x0, we0)
        h1_t[1] = emit_S2_mm1(1, x1, we1)
        x_t = {0: x0, 1: x1}
        we_t = {0: we0, 1: we1}

        # W2: fp32 staging -> bf16 on DVE (runs while PE does mm1(0)/mm1(1))
        w2_sb = cons.tile([P, DC, D], BF16)
        for g in range(4):
            wtmp = stage.tile([P, MC, D], F32, tag="stage", name=f"w2t{g}")
            nc.sync.dma_start(
                wtmp, w2_d[g * M:(g + 1) * M, :].rearrange("(c p) d -> p c d", p=P)
            )
            nc.vector.tensor_copy(w2_sb[:, 2 * g:2 * g + 2, :], wtmp)

        # values b0 + keys b2 DMA issue
        vals_f = vpool.tile([P, TC, D], F32, tag="vf", name="vals_f")
        nc.sync.dma_start(vals_f, v_d[0].rearrange("(to p) d -> p to d", p=P))
        kst[2] = stage.tile([P, TC, M], F32, tag="stage", name="kst2")
        nc.sync.dma_start(kst[2], k_d[2].rearrange("(to p) m -> p to m", p=P))

        # ---- main pipeline: region R(b) covers mm2(b) on the PE -------------
        for b in range(B):
            # keys(b+2) cast + mask(b+2) prefetch
            if b + 2 < B:
                keys_bf[b + 2] = kbf.tile(
                    [P, TC, M], BF16, tag="kb", name=f"kbf{b + 2}"
                )
                nc.vector.tensor_copy(keys_bf[b + 2], kst[b + 2])
                mask_t[b + 2] = small.tile([1, T], F32, tag="mask", name=f"mask{b + 2}")
                nc.sync.dma_start(mask_t[b + 2], m_d[b + 2:b + 3, :])

            # mm2(b) + score (DVE FMA chain; last batch scores on the PE
            # so the tail has no long vector chain). attn@values for b-1 is
            # interleaved at j==2 (its softmax bounce lands by then).
            last = b == B - 1
            if last:
                score_ps = [
                    psum_sc.tile([1, 512], F32, tag="sc", name=f"sps{b}_{h}")
                    for h in range(NH)
                ]
            sacc = [None, None]
            for j in range(DC):
                h2 = h2pool.tile([P, T], BF16, tag="H2", name=f"h2_{b}_{j}")
                for h in range(NH):
                    ps = psum_mm.tile([P, 512], F32, tag="mm")
                    for c in range(DC):
                        nc.tensor.matmul(
                            ps, w2_sb[:, c, j * P:(j + 1) * P], h1_t[b][:, c, _ns(h)],
                            start=(c == 0), stop=(c == DC - 1),
                        )
                    nc.scalar.activation(
                        h2[:, _ns(h)], ps, AF.Relu, bias=b2_sb[:, j:j + 1], scale=1.0
                    )
                if j == 2 and b >= 1:
                    emit_attnv_pe(b - 1)
                if last:
                    for h in range(NH):
                        nc.tensor.matmul(
                            score_ps[h], ws_sb[:, j:j + 1], h2[:, _ns(h)],
                            start=(j == 0), stop=(j == DC - 1),
                            skip_group_check=True,
                        )
                    continue
                ch = j % 2
                sa = accp.tile([P, T], BF16, tag="acc", name=f"sc{b}_{j}")
                if j < 2:
                    nc.vector.tensor_scalar_mul(sa, h2, ws_f[:, j:j + 1])
                else:
                    nc.vector.scalar_tensor_tensor(
                        sa, in0=h2, scalar=ws_f[:, j:j + 1], in1=sacc[ch],
                        op0=OP.mult, op1=OP.add,
                    )
                sacc[ch] = sa
            if not last:
                saf = accp.tile([P, T], BF16, tag="acc", name=f"scf{b}")
                nc.vector.tensor_add(saf, sacc[0], sacc[1])

            # score reduce + softmax
            if not last:
                score_ps = [
                    psum_sc.tile([1, 512], F32, tag="sc", name=f"sps{b}_{h}")
                    for h in range(NH)
                ]
                for h in range(NH):
                    nc.tensor.matmul(
                        score_ps[h], ones_col, saf[:, _ns(h)], start=True, stop=True
                    )

            score_sb = small1.tile([1, T], F32, tag="ssb", name=f"ssb{b}")
            for h in range(NH):
                nc.vector.scalar_tensor_tensor(
                    score_sb[:, _ns(h)], in0=mask_t[b][:, _ns(h)], scalar=NEG,
                    in1=score_ps[h], op0=OP.mult, op1=OP.add,
                )
            sum_sb = small1.tile([1, 1], F32, tag="sum", name=f"sum{b}")
            exp_f = small1.tile([1, T], BF16, tag="expf", name=f"expf{b}")
            nc.scalar.activation(exp_f, score_sb, AF.Exp, accum_out=sum_sb)
            rec = small.tile([1, 1], F32, tag="rec", name=f"rec{b}")
            nc.vector.reciprocal(rec, sum_sb)
            # attn_t[p, c] = exp_score[c*128 + p]
            attn_t = small.tile([P, TC], BF16, tag="attn", name=f"attn{b}")
            if last:
                # tail is latency-bound: transpose exp on the PE via 8 tiny
                # matmuls (lhsT = exp chunk row, rhs = [1]-ones)
                attn_ps = psum_out.tile([P, TC], F32, tag="po", name=f"aps{b}")
                for c in range(TC):
                    nc.tensor.matmul(
                        attn_ps[:, c:c + 1], exp_f[0:1, c * P:(c + 1) * P],
                        ones_col[0:1, 0:1], start=True, stop=True,
                        skip_group_check=True,
                    )
                nc.vector.tensor_copy(attn_t, attn_ps)
            else:
                attn_dram = dram.tile([1, T], BF16, tag="ad", name=f"ad{b}")
                nc.scalar.dma_start(attn_dram, exp_f)
                nc.sync.dma_start(
                    attn_t, attn_dram.rearrange("one (c p) -> p (one c)", p=P)
                )

            # values(b) cast (scalar; DMA landed during this region) and
            # values(b+1) + keys(b+3) DMA issue
            vals_bf = vbpool.tile([P, TC, D], BF16, tag="vb", name=f"vb{b}")
            for k in range(4):
                nc.scalar.activation(
                    vals_bf[:, 2 * k:2 * k + 2, :], vals_f[:, 2 * k:2 * k + 2, :],
                    AF.Copy, bias=0.0, scale=1.0,
                )
            carry[b] = {"attn": attn_t, "vals": vals_bf, "rec": rec}
            if b + 1 < B:
                vals_f = vpool.tile([P, TC, D], F32, tag="vf", name=f"vals_f{b + 1}")
                nc.sync.dma_start(
                    vals_f, v_d[b + 1].rearrange("(to p) d -> p to d", p=P)
                )
            if b + 3 < B:
                kst[b + 3] = stage.tile([P, TC, M], F32, tag="stage", name=f"kst{b + 3}")
                nc.sync.dma_start(
                    kst[b + 3], k_d[b + 3].rearrange("(to p) m -> p to m", p=P)
                )

            # transposes (xbar DMAs) + w1eff + mm1 for b+2
            # (PE runs one batch ahead of mm2)
            if b + 2 < B:
                x_t[b + 2] = emit_S1_transp(b + 2)
                we_t[b + 2] = emit_S1_weff(b + 2)
                h1_t[b + 2] = emit_S2_mm1(b + 2, x_t[b + 2], we_t[b + 2])

        # tail: last batch's attn@values
        emit_attnv_pe(B - 1)

    nc.compile()
    return nc


def _get_built():
    global _built
    if _built is None:
        _built = _build()
    return _built


N_CORES = 8


def make_in_maps(query, keys, values, mask, W1, b1, W2, b2, w_score, b_score=None):
    query = np.ascontiguousarray(np.asarray(query, dtype=np.float32).reshape(64, M))
    keys = np.ascontiguousarray(np.asarray(keys, dtype=np.float32))
    values = np.ascontiguousarray(np.asarray(values, dtype=np.float32))
    mask = np.ascontiguousarray(np.asarray(mask, dtype=np.float32).reshape(64, T))
    shared = {
        "W1": np.ascontiguousarray(np.asarray(W1, dtype=np.float32)),
        "b1": np.ascontiguousarray(np.asarray(b1, dtype=np.float32)),
        "W2": np.ascontiguousarray(np.asarray(W2, dtype=np.float32)),
        "b2": np.ascontiguousarray(np.asarray(b2, dtype=np.float32)),
        "w_score": np.ascontiguousarray(np.asarray(w_score, dtype=np.float32)),
    }
    in_maps = []
    for c in range(N_CORES):
        sl = slice(c * B, (c + 1) * B)
        in_maps.append({
            "query": query[sl],
            "keys": keys[sl],
            "values": values[sl],
            "mask": mask[sl],
            **shared,
        })
    return in_maps


def gather_out(results):
    out = np.concatenate([results[c]["out"] for c in range(N_CORES)], axis=0)
    return out.reshape(64, 1, D).astype(np.float32)


def kernel(query, keys, values, mask, W1, b1, W2, b2, w_score, b_score):
    """Full-input entry point: shards over 8 NeuronCores, returns [64, 1, D]."""
    from concourse.bass_utils import run_bass_kernel_spmd

    nc = _get_built()
    in_maps = make_in_maps(query, keys, values, mask, W1, b1, W2, b2, w_score)
    res = run_bass_kernel_spmd(nc, in_maps, core_ids=list(range(N_CORES)))
    return gather_out(res.results)


# revision 12
# speedup vs baseline: 1.2136x; 1.2136x over previous
"""DIN attention layer kernel for Trainium2 (8 NeuronCores, data-parallel over batch).

Reference computation (per batch b):
    att = [q, k, q-k, q*k]            # [T, 4M]
    h1  = relu(att @ W1 + b1)         # [T, D]
    h2  = relu(h1 @ W2 + b2)          # [T, D]
    s   = h2 @ w_score + b_score      # [T, 1]
    attn = softmax(s.T + mask * -1e9) # [1, T]
    out = attn @ values               # [1, D]

Key optimizations over the straightforward mapping:
  * Data-parallel: 8 batches per core (B=64 over 8 cores).
  * Algebraic reassociation of the concat matmul:
        att @ W1 = q@(W1a+W1c) + k@(W1b-W1c+diag(q)@W1d)
    The q term is a per-batch row vector folded into the layer-1 bias (rt);
    diag(q)@W1d is folded into the layer-1 weight per batch (w1eff, two DVE
    FMAs), so the mm1 contraction is 256 instead of 1024 (mm1 quartered).
  * Transposed-activation layout [feature, token]: W1/W2 are used as stored
    for lhsT; only keys need a transpose, done on the PE via bf16 identity
    matmul (keys pre-cast to bf16; 4 transposes share one PSUM tile).
  * bf16 matmuls (fp32 PSUM accumulation); biases/softmax kept in fp32.
  * score = w_score . h2 and out = attn @ values are computed as
    per-partition FMA chains on the Vector engine (scalar_tensor_tensor)
    followed by a single ones-vector reduction matmul, instead of
    streaming h2/values through the PE a second time.
  * Softmax without max-subtraction (scores are O(1); masked lanes are
    exp(-1e9) = 0), sum fused into the Exp activation via accum_out.
  * Software pipelined emission: mm1 runs one batch ahead of mm2; the
    deferred attn@values chain for batch b runs during batch b+1's mm2.
"""

import numpy as np

P = 128
B = 8          # batches per core
T = 1024       # tokens
M = 256        # key feature dim
D = 1024       # hidden dim
MC = M // P    # key-feature chunks (2)
DC = D // P    # hidden chunks (8)
TC = T // P    # token chunks (8)
NH = 2         # free-dim halves of 512
NEG = -1.0e9

_built = None


def _ns(h):
    return slice(h * 512, (h + 1) * 512)


def _build():
    import concourse.bass as bass
    import concourse.bacc as bacc
    import concourse.mybir as mybir
    import concourse.tile as tile
    from concourse.masks import make_identity
    from contextlib import ExitStack

    F32 = mybir.dt.float32
    BF16 = mybir.dt.bfloat16
    AF = mybir.ActivationFunctionType
    OP = mybir.AluOpType

    nc = bacc.Bacc("TRN2")
    q_d = nc.dram_tensor("query", [B, M], F32, kind="ExternalInput").ap()
    k_d = nc.dram_tensor("keys", [B, T, M], F32, kind="ExternalInput").ap()
    v_d = nc.dram_tensor("values", [B, T, D], F32, kind="ExternalInput").ap()
    m_d = nc.dram_tensor("mask", [B, T], F32, kind="ExternalInput").ap()
    w1_d = nc.dram_tensor("W1", [4 * M, D], F32, kind="ExternalInput").ap()
    b1_d = nc.dram_tensor("b1", [D], F32, kind="ExternalInput").ap()
    w2_d = nc.dram_tensor("W2", [D, D], F32, kind="ExternalInput").ap()
    b2_d = nc.dram_tensor("b2", [D], F32, kind="ExternalInput").ap()
    ws_d = nc.dram_tensor("w_score", [D, 1], F32, kind="ExternalInput").ap()
    out_d = nc.dram_tensor("out", [B, D], F32, kind="ExternalOutput").ap()

    with tile.TileContext(nc) as tc, ExitStack() as ctx:
        cons = ctx.enter_context(tc.tile_pool(name="cons", bufs=1))
        stage = ctx.enter_context(tc.tile_pool(name="stage", bufs=4))
        kbf = ctx.enter_context(tc.tile_pool(name="kbf", bufs=2))
        xpool = ctx.enter_context(tc.tile_pool(name="xp", bufs=2))
        wep = ctx.enter_context(tc.tile_pool(name="wep", bufs=2))
        h1pool = ctx.enter_context(tc.tile_pool(name="h1p", bufs=2))
        h2pool = ctx.enter_context(tc.tile_pool(name="h2p", bufs=3))
        vpool = ctx.enter_context(tc.tile_pool(name="vp", bufs=1))
        vbpool = ctx.enter_context(tc.tile_pool(name="vbp", bufs=1))
        accp = ctx.enter_context(tc.tile_pool(name="accp", bufs=4))
        small = ctx.enter_context(tc.tile_pool(name="small", bufs=2))
        small1 = ctx.enter_context(tc.tile_pool(name="small1", bufs=1))
        dram = ctx.enter_context(tc.tile_pool(name="dram", bufs=2, space="DRAM"))
        psum_mm = ctx.enter_context(tc.tile_pool(name="psmm", bufs=4, space="PSUM"))
        psum_sc = ctx.enter_context(tc.tile_pool(name="pssc", bufs=2, space="PSUM"))
        psum_out = ctx.enter_context(tc.tile_pool(name="psout", bufs=2, space="PSUM"))

        # ---- constants ------------------------------------------------------
        identity = cons.tile([P, P], BF16)
        make_identity(nc, identity)
        ones_col = cons.tile([P, 1], BF16)
        nc.vector.memset(ones_col, 1.0)

        # ---- bulk input DMA issue, priority order (sync queue) --------------
        # keys b0/b1 first (feed the first PE work), then W1 pieces
        # (c, b, d gate mm1; a gates the rt bias), then W2, values b0, keys b2.
        kst = {}
        kst[0] = stage.tile([P, TC, M], F32, tag="stage", name="kst0")
        nc.sync.dma_start(kst[0], k_d[0].rearrange("(to p) m -> p to m", p=P))
        tmp_c = stage.tile([P, MC, D], F32, tag="stage", name="w1ct")
        nc.sync.dma_start(
            tmp_c, w1_d[2 * M:3 * M, :].rearrange("(c p) d -> p c d", p=P)
        )
        tmp_a = stage.tile([P, MC, D], F32, tag="stage", name="w1at")
        nc.sync.dma_start(tmp_a, w1_d[0:M, :].rearrange("(c p) d -> p c d", p=P))
        tmp_b = stage.tile([P, MC, D], F32, tag="stage", name="w1bt")
        nc.sync.dma_start(tmp_b, w1_d[M:2 * M, :].rearrange("(c p) d -> p c d", p=P))
        kst[1] = stage.tile([P, TC, M], F32, tag="stage", name="kst1")
        nc.sync.dma_start(kst[1], k_d[1].rearrange("(to p) m -> p to m", p=P))
        tmp_d = stage.tile([P, MC, D], F32, tag="stage", name="w1dt")
        nc.sync.dma_start(
            tmp_d, w1_d[3 * M:4 * M, :].rearrange("(c p) d -> p c d", p=P)
        )
        mask_t = {}
        for b in range(2):
            mask_t[b] = small.tile([1, T], F32, tag="mask", name=f"mask{b}")
            nc.sync.dma_start(mask_t[b], m_d[b:b + 1, :])

        # striped per-channel vectors via gpsimd (software DGE, parallel queue)
        qt_f = cons.tile([P, MC, B], F32)
        for c in range(MC):
            nc.gpsimd.dma_start(
                qt_f[:, c, :], q_d[:, c * P:(c + 1) * P].rearrange("b p -> p b")
            )
        b1_sb = cons.tile([P, DC], F32)
        nc.gpsimd.dma_start(b1_sb, b1_d.rearrange("(c p) -> p c", p=P))
        b2_sb = cons.tile([P, DC], F32)
        nc.gpsimd.dma_start(b2_sb, b2_d.rearrange("(c p) -> p c", p=P))
        ws_f = cons.tile([P, DC], F32)
        nc.gpsimd.dma_start(ws_f, ws_d.rearrange("(c p) one -> p (c one)", p=P))
        ws_sb = cons.tile([P, DC], BF16)
        nc.vector.tensor_copy(ws_sb, ws_f)

        # keys b0 cast (first DVE op so transposes can start immediately)
        keys_bf = {}
        keys_bf[0] = kbf.tile([P, TC, M], BF16, tag="kb", name="kbf0")
        nc.vector.tensor_copy(keys_bf[0], kst[0])

        # ---- per-batch stage emitters ---------------------------------------

        def emit_S1_transp(b):
            """keys^T via PE: x_t[p, c, t] = keys[b, t, c*128+p] (bf16)."""
            x_t = xpool.tile([P, MC, T], BF16, tag="X", name=f"x{b}")
            for c in range(MC):
                tp = psum_mm.tile([P, T], BF16, tag="mm", name=f"tp{b}_{c}")
                for to in range(TC):
                    nc.tensor.transpose(
                        tp[:, to * P:(to + 1) * P],
                        keys_bf[b][:, to, c * P:(c + 1) * P],
                        identity,
                    )
                nc.vector.tensor_copy(x_t[:, c, :], tp)
            return x_t

        def emit_S1_weff(b):
            """w1eff_b = (W1b - W1c) + q_b * W1d  (per-batch layer-1 weight)."""
            we = wep.tile([P, MC, D], BF16, tag="we", name=f"we{b}")
            for c in range(MC):
                nc.vector.scalar_tensor_tensor(
                    we[:, c, :], in0=w1d_sb[:, c, :], scalar=qt_f[:, c, b:b + 1],
                    in1=w1bc[:, c, :], op0=OP.mult, op1=OP.add,
                )
            return we

        def emit_S2_mm1(b, x_t, we):
            """h1 = relu(w1eff^T @ x + rt_b)  -> [P, DC, T] bf16."""
            h1 = h1pool.tile([P, DC, T], BF16, tag="H1", name=f"h1_{b}")
            for j in range(DC):
                for h in range(NH):
                    ps = psum_mm.tile([P, 512], F32, tag="mm")
                    for c in range(MC):
                        nc.tensor.matmul(
                            ps, we[:, c, j * P:(j + 1) * P], x_t[:, c, _ns(h)],
                            start=(c == 0), stop=(c == MC - 1),
                        )
                    nc.scalar.activation(
                        h1[:, j, _ns(h)], ps, AF.Relu, bias=rt[:, b, j:j + 1],
                        scale=1.0,
                    )
            return h1

        carry = {}

        def emit_attnv_pe(b):
            """out = (attn_t^T @ vals) * rec on the PE (16 bf16 matmuls)."""
            st = carry.pop(b)
            out_sb = small1.tile([1, D], F32, tag="osb", name=f"osb{b}")
            ops = [
                psum_out.tile([1, 512], F32, tag="po", name=f"ops{b}_{h}")
                for h in range(NH)
            ]
            for h in range(NH):
                for c in range(TC):
                    nc.tensor.matmul(
                        ops[h], st["attn"][:, c:c + 1], st["vals"][:, c, _ns(h)],
                        start=(c == 0), stop=(c == TC - 1),
                    )
            for h in range(NH):
                nc.vector.tensor_scalar_mul(out_sb[:, _ns(h)], ops[h], st["rec"])
            nc.sync.dma_start(out_d[b:b + 1, :], out_sb)

        # ---- preamble: transposes, rt, then mm1 for batches 0 and 1 ---------
        x0 = emit_S1_transp(0)

        # rt[p, b, j] = (q_b @ (W1a+W1c))[j*128+p] + b1[j*128+p]; MUST be
        # fully emitted before any h1 activation that reads it.
        w1qc = cons.tile([P, MC, D], BF16)   # W1a + W1c
        nc.vector.tensor_add(w1qc, tmp_a, tmp_c)
        qt_b = cons.tile([P, MC, B], BF16)
        nc.vector.tensor_copy(qt_b, qt_f)
        rt = cons.tile([P, B, DC], F32)
        for j in range(DC):
            rt_ps = psum_sc.tile([P, B], F32, tag="sc", name=f"rtps{j}")
            for c in range(MC):
                nc.tensor.matmul(
                    rt_ps, w1qc[:, c, j * P:(j + 1) * P], qt_b[:, c, :],
                    start=(c == 0), stop=(c == MC - 1),
                )
            nc.vector.tensor_scalar(
                rt[:, :, j], rt_ps, b1_sb[:, j:j + 1], None, op0=OP.add
            )

        # keys b1 cast + transposes
        keys_bf[1] = kbf.tile([P, TC, M], BF16, tag="kb", name="kbf1")
        nc.vector.tensor_copy(keys_bf[1], kst[1])
        x1 = emit_S1_transp(1)

        # W1 combos for mm1 (DVE; ordered by DMA arrival)
        w1bc = cons.tile([P, MC, D], BF16)   # W1b - W1c
        nc.vector.tensor_sub(w1bc, tmp_b, tmp_c)
        w1d_sb = cons.tile([P, MC, D], BF16)  # W1d
        nc.vector.tensor_copy(w1d_sb, tmp_d)
        we0 = emit_S1_weff(0)
        we1 = emit_S1_weff(1)

        h1_t = {}
        h1_t[0] = emit_S2_mm1(0, x0, we0)
        h1_t[1] = emit_S2_mm1(1, x1, we1)
        x_t = {0: x0, 1: x1}
        we_t = {0: we0, 1: we1}

        # W2: fp32 staging -> bf16 on DVE (runs while PE does mm1(0)/mm1(1))
        w2_sb = cons.tile([P, DC, D], BF16)
        for g in range(4):
            wtmp = stage.tile([P, MC, D], F32, tag="stage", name=f"w2t{g}")
            nc.sync.dma_start(
                wtmp, w2_d[g * M:(g + 1) * M, :].rearrange("(c p) d -> p c d", p=P)
            )
            nc.vector.tensor_copy(w2_sb[:, 2 * g:2 * g + 2, :], wtmp)

        # values b0 + keys b2 DMA issue
        vals_f = vpool.tile([P, TC, D], F32, tag="vf", name="vals_f")
        nc.sync.dma_start(vals_f, v_d[0].rearrange("(to p) d -> p to d", p=P))
        kst[2] = stage.tile([P, TC, M], F32, tag="stage", name="kst2")
        nc.sync.dma_start(kst[2], k_d[2].rearrange("(to p) m -> p to m", p=P))

        # ---- main pipeline: region R(b) covers mm2(b) on the PE -------------
        for b in range(B):
            # keys(b+2) cast + mask(b+2) prefetch
            if b + 2 < B:
                keys_bf[b + 2] = kbf.tile(
                    [P, TC, M], BF16, tag="kb", name=f"kbf{b + 2}"
                )
                nc.vector.tensor_copy(keys_bf[b + 2], kst[b + 2])
                mask_t[b + 2] = small.tile([1, T], F32, tag="mask", name=f"mask{b + 2}")
                nc.sync.dma_start(mask_t[b + 2], m_d[b + 2:b + 3, :])

            # mm2(b) + score (DVE FMA chain; last batch scores on the PE
            # so the tail has no long vector chain). attn@values for b-1 is
            # interleaved at j==2 (its softmax bounce lands by then).
            last = b == B - 1
            if last:
                score_ps = [
                    psum_sc.tile([1, 512], F32, tag="sc", name=f"sps{b}_{h}")
                    for h in range(NH)
                ]
            sacc = [None, None]
            for j in range(DC):
                h2 = h2pool.tile([P, T], BF16, tag="H2", name=f"h2_{b}_{j}")
                for h in range(NH):
                    ps = psum_mm.tile([P, 512], F32, tag="mm")
                    for c in range(DC):
                        nc.tensor.matmul(
                            ps, w2_sb[:, c, j * P:(j + 1) * P], h1_t[b][:, c, _ns(h)],
                            start=(c == 0), stop=(c == DC - 1),
                        )
                    nc.scalar.activation(
                        h2[:, _ns(h)], ps, AF.Relu, bias=b2_sb[:, j:j + 1], scale=1.0
                    )
                if j == 2 and b >= 1:
                    emit_attnv_pe(b - 1)
                if last:
                    for h in range(NH):
                        nc.tensor.matmul(
                            score_ps[h], ws_sb[:, j:j + 1], h2[:, _ns(h)],
                            start=(j == 0), stop=(j == DC - 1),
                            skip_group_check=True,
                        )
                    continue
                ch = j % 2
                sa = accp.tile([P, T], BF16, tag="acc", name=f"sc{b}_{j}")
                if j < 2:
                    nc.vector.tensor_scalar_mul(sa, h2, ws_f[:, j:j + 1])
                else:
                    nc.vector.scalar_tensor_tensor(
                        sa, in0=h2, scalar=ws_f[:, j:j + 1], in1=sacc[ch],
                        op0=OP.mult, op1=OP.add,
                    )
                sacc[ch] = sa
            if not last:
                saf = accp.tile([P, T], BF16, tag="acc", name=f"scf{b}")
                nc.vector.tensor_add(saf, sacc[0], sacc[1])

            # transposes + w1eff for b+2 (PE right after mm2, psums drain late)
            if b + 2 < B:
                x_t[b + 2] = emit_S1_transp(b + 2)
                we_t[b + 2] = emit_S1_weff(b + 2)

            # score reduce + softmax
            if not last:
                score_ps = [
                    psum_sc.tile([1, 512], F32, tag="sc", name=f"sps{b}_{h}")
                    for h in range(NH)
                ]
                for h in range(NH):
                    nc.tensor.matmul(
                        score_ps[h], ones_col, saf[:, _ns(h)], start=True, stop=True
                    )

            score_sb = small1.tile([1, T], F32, tag="ssb", name=f"ssb{b}")
            for h in range(NH):
                nc.vector.scalar_tensor_tensor(
                    score_sb[:, _ns(h)], in0=mask_t[b][:, _ns(h)], scalar=NEG,
                    in1=score_ps[h], op0=OP.mult, op1=OP.add,
                )
            sum_sb = small1.tile([1, 1], F32, tag="sum", name=f"sum{b}")
            exp_f = small1.tile([1, T], BF16, tag="expf", name=f"expf{b}")
            nc.scalar.activation(exp_f, score_sb, AF.Exp, accum_out=sum_sb)
            rec = small.tile([1, 1], F32, tag="rec", name=f"rec{b}")
            nc.vector.reciprocal(rec, sum_sb)
            # attn_t[p, c] = exp_score[c*128 + p] via a DRAM bounce
            attn_dram = dram.tile([1, T], BF16, tag="ad", name=f"ad{b}")
            nc.scalar.dma_start(attn_dram, exp_f)
            attn_t = small.tile([P, TC], BF16, tag="attn", name=f"attn{b}")
            nc.sync.dma_start(
                attn_t, attn_dram.rearrange("one (c p) -> p (one c)", p=P)
            )

            # values(b) cast (scalar; DMA landed during this region) and
            # values(b+1) + keys(b+3) DMA issue
            vals_bf = vbpool.tile([P, TC, D], BF16, tag="vb", name=f"vb{b}")
            for k in range(4):
                nc.scalar.activation(
                    vals_bf[:, 2 * k:2 * k + 2, :], vals_f[:, 2 * k:2 * k + 2, :],
                    AF.Copy, bias=0.0, scale=1.0,
                )
            carry[b] = {"attn": attn_t, "vals": vals_bf, "rec": rec}
            if b + 1 < B:
                vals_f = vpool.tile([P, TC, D], F32, tag="vf", name=f"vals_f{b + 1}")
                nc.sync.dma_start(
                    vals_f, v_d[b + 1].rearrange("(to p) d -> p to d", p=P)
                )
            if b + 3 < B:
                kst[b + 3] = stage.tile([P, TC, M], F32, tag="stage", name=f"kst{b + 3}")
                nc.sync.dma_start(
                    kst[b + 3], k_d[b + 3].rearrange("(to p) m -> p to m", p=P)
                )

            # mm1 for b+2 (PE runs one batch ahead of mm2)
            if b + 2 < B:
                h1_t[b + 2] = emit_S2_mm1(b + 2, x_t[b + 2], we_t[b + 2])

        # tail: last batch's attn@values
        emit_attnv_pe(B - 1)

    nc.compile()
    return nc


def _get_built():
    global _built
    if _built is None:
        _built = _build()
    return _built


N_CORES = 8


def make_in_maps(query, keys, values, mask, W1, b1, W2, b2, w_score, b_score=None):
    query = np.ascontiguousarray(np.asarray(query, dtype=np.float32).reshape(64, M))
    keys = np.ascontiguousarray(np.asarray(keys, dtype=np.float32))
    values = np.ascontiguousarray(np.asarray(values, dtype=np.float32))
    mask = np.ascontiguousarray(np.asarray(mask, dtype=np.float32).reshape(64, T))
    shared = {
        "W1": np.ascontiguousarray(np.asarray(W1, dtype=np.float32)),
        "b1": np.ascontiguousarray(np.asarray(b1, dtype=np.float32)),
        "W2": np.ascontiguousarray(np.asarray(W2, dtype=np.float32)),
        "b2": np.ascontiguousarray(np.asarray(b2, dtype=np.float32)),
        "w_score": np.ascontiguousarray(np.asarray(w_score, dtype=np.float32)),
    }
    in_maps = []
    for c in range(N_CORES):
        sl = slice(c * B, (c + 1) * B)
        in_maps.append({
            "query": query[sl],
            "keys": keys[sl],
            "values": values[sl],
            "mask": mask[sl],
            **shared,
        })
    return in_maps


def gather_out(results):
    out = np.concatenate([results[c]["out"] for c in range(N_CORES)], axis=0)
    return out.reshape(64, 1, D).astype(np.float32)


def kernel(query, keys, values, mask, W1, b1, W2, b2, w_score, b_score):
    """Full-input entry point: shards over 8 NeuronCores, returns [64, 1, D]."""
    from concourse.bass_utils import run_bass_kernel_spmd

    nc = _get_built()
    in_maps = make_in_maps(query, keys, values, mask, W1, b1, W2, b2, w_score)
    res = run_bass_kernel_spmd(nc, in_maps, core_ids=list(range(N_CORES)))
    return gather_out(res.results)
